# revision 2
# baseline (speedup 1.0000x reference)
"""Trainium2 Bass kernel for the detection-loss problem — v2 (fp16).

Data-parallel over batch: each of 8 NeuronCores processes one image.

Key design vs v1:
  * fp16 everywhere on the pair stage -> DVE 2x packed mode.
  * pair layout [128p, (m=32 boxes outer, f=64 anchors inner)]: every
    broadcast operand ([p, f]-shaped anchor columns, box constants
    replicated host-side) is step-1 in the innermost dim, keeping 2x.
  * ln-space IoU score (monotone): lq = Ln(ir) - Ln(areaq + areaB/4),
    thresholds ln(0.8), ln(4/11). Scalar engine does the two Ln's.
  * argmax one-hot mask -> strided-view 32x32 DVE block transpose gives
    tmask[m-on-partitions, (f, i)] directly; payload gather is 64
    fp16 matmuls/tile (32-row weights, 128-col out) instead of v1's 96
    fp32 (2-pass) 32x32 matmuls; matched-label logits via 16 batched
    [128x128]x[128x88] matmuls/tile accumulating a G[(fr,m),(fr,c)]
    PSUM block (host sums the 4 diagonal blocks).
  * payload split hi/lo fp16 for full coordinate precision; argmax ties
    (~0.3% of anchors) are averaged via a reciprocal of the gathered
    mask-count.
  * per-anchor lse/cls0 stored per tile; single Ln + 3 accumulating
    ops at the end (keeps scalar-engine activation-table swaps at 2/tile).

Anchor indexing: n = t*8192 + p*64 + f  (t tile, p partition, f free),
N 90000 padded to NT*128*F = 90112 with valid=0 rows.
Reg/gather work runs in the matmul-output permuted layout:
  chunk CH = pb*16 + fc, out partition q = f_rel*32 + i:
  anchor n = t*8192 + (pb*32 + i)*64 + (4*fc + f_rel).
"""

import numpy as np

B, A, C, H, W = 8, 9, 21, 100, 100
M = 32
N = A * H * W            # 90000
F = 64                   # anchors per partition per tile
NT = 11
TPB = 128 * F            # 8192
NPAD = NT * TPB          # 90112
NTF = NT * F             # 704
C2 = 22                  # classes padded to 22 for alignment
KQ = 10                  # gather payload columns (hi4, lo4, ones, pad)
LN_POS = float(np.log(0.8))       # iou >= 0.25  <=>  lq >= ln(0.8)
LN_NEG = float(np.log(4.0 / 11.0))  # iou < 0.1  <=>  lq < ln(4/11)
CLAMP = 6.1e-5

# staging columns
SC_NP, SC_NN, SC_SL = 0, NT, 2 * NT
SC_PLSE, SC_NLSE, SC_NCLS0 = 3 * NT, 3 * NT + 1, 3 * NT + 2
SC_TOT = 3 * NT + 3      # 36

_CACHE = {}
LAST_RESULTS = None

f16n = np.float16
f32n = np.float32


def _plane(v):
    """[NPAD] -> [128, NTF] tiling, n = t*TPB + p*F + f."""
    return v.reshape(NT, 128, F).transpose(1, 0, 2).reshape(128, NTF)


def _pad(col, pv):
    full = np.full(NPAD, pv, np.float64)
    full[:N] = col
    return full


def _nmap():
    """[128, NT, 4, 16] anchor index for permuted (matmul-output) layout."""
    q = np.arange(128)[:, None, None, None]
    t = np.arange(NT)[None, :, None, None]
    pb = np.arange(4)[None, None, :, None]
    fc = np.arange(16)[None, None, None, :]
    return t * TPB + (pb * 32 + q % 32) * F + 4 * fc + q // 32


def _host_prep_shared(anchors):
    anchors = np.asarray(anchors, np.float64)
    aw = anchors[:, 2] - anchors[:, 0]
    ah = anchors[:, 3] - anchors[:, 1]
    acx = anchors[:, 0] + 0.5 * aw
    acy = anchors[:, 1] + 0.5 * ah

    acap = np.concatenate([
        _plane(_pad(aw / 2, 1.0)), _plane(_pad(acx - aw / 4, -4000.0)),
        _plane(_pad(ah / 2, 1.0)), _plane(_pad(acy - ah / 4, -4000.0)),
    ], axis=1).astype(f16n).copy()
    avv = _plane(_pad(np.ones(N), 0.0)).astype(f16n).copy()

    s2k = np.stack([_pad(2 / aw, 1.0), _pad(2 / ah, 1.0),
                    _pad(np.ones(N), 1.0), _pad(np.ones(N), 1.0)], 1)
    bk = np.stack([_pad(0.5 - 2 * acx / aw, 0.0), _pad(0.5 - 2 * acy / ah, 0.0),
                   _pad(-np.log(aw), 0.0), _pad(-np.log(ah), 0.0)], 1)
    nm = _nmap()
    s2p = s2k[nm].reshape(128, NT * 256).astype(f16n).copy()
    return acap, avv, s2p, bk, nm


def _host_prep_image(cls_i, reg_i, tb_i, nm, bk):
    cls_flat = np.transpose(np.asarray(cls_i, np.float64), (0, 2, 3, 1)).reshape(N, C)
    reg_flat = np.transpose(np.asarray(reg_i, np.float64), (0, 2, 3, 1)).reshape(N, 4)

    regp = np.zeros((NPAD, 4))
    regp[:N] = reg_flat
    regb = np.concatenate([_plane(regp[:, k]) for k in range(4)], 1).astype(f16n).copy()
    r2bp = (regp - bk)[nm].reshape(128, NT * 256).astype(f16n).copy()

    clsp = np.full((NPAD, C2), -60000.0)
    clsp[:N, :C] = cls_flat
    clsp[N:, :C] = 0.0
    clsb = clsp.reshape(NT, 128, F * C2).transpose(1, 0, 2).reshape(128, -1)
    clsb = clsb.astype(f16n).copy()

    tb = np.asarray(tb_i, np.float64)
    bw = tb[:, 2] - tb[:, 0]
    bh = tb[:, 3] - tb[:, 1]
    planes = [tb[:, 2], tb[:, 0], tb[:, 3], tb[:, 1], bw * bh / 4]
    brep = np.concatenate(
        [np.tile(np.repeat(p, F), (128, 1)) for p in planes], 1).astype(f16n).copy()

    pay = np.stack([tb[:, 0] + bw / 2, tb[:, 1] + bh / 2,
                    np.log(bw), np.log(bh)], 1)       # [M, 4]
    hi = pay.astype(f16n)
    lo = (pay - hi.astype(np.float64)).astype(f16n)
    p10 = np.concatenate([hi, lo, np.ones((M, 1), f16n),
                          np.zeros((M, 1), f16n)], 1)  # [32, 10]
    ptab = np.tile(p10, (4, 1)).astype(f16n).copy()    # [128, 10]
    return regb, r2bp, clsb, brep, ptab


def _build_nc():
    import concourse.bacc as bacc
    import concourse.mybir as mybir
    from concourse.tile import TileContext

    dt = mybir.dt
    f16 = dt.float16
    f32 = dt.float32
    op = mybir.AluOpType
    act = mybir.ActivationFunctionType
    X = mybir.AxisListType.X

    nc = bacc.Bacc("TRN2", target_bir_lowering=False, debug=False, num_devices=8)

    regb_d = nc.dram_tensor("regb", [128, 4 * NTF], f16, kind="ExternalInput")
    acap_d = nc.dram_tensor("acap", [128, 4 * NTF], f16, kind="ExternalInput")
    avv_d = nc.dram_tensor("avv", [128, NTF], f16, kind="ExternalInput")
    brep_d = nc.dram_tensor("brep", [128, 5 * M * F], f16, kind="ExternalInput")
    clsb_d = nc.dram_tensor("clsb", [128, NTF * C2], f16, kind="ExternalInput")
    ptab_d = nc.dram_tensor("ptab", [128, KQ], f16, kind="ExternalInput")
    r2b_d = nc.dram_tensor("r2b", [128, NT * 256], f16, kind="ExternalInput")
    s2_d = nc.dram_tensor("s2", [128, NT * 256], f16, kind="ExternalInput")
    stage_d = nc.dram_tensor("stage", [128, SC_TOT], f32, kind="ExternalOutput")
    gmat_d = nc.dram_tensor("gmat", [128, 128], f32, kind="ExternalOutput")

    PMF = [128, M, F]

    with nc.allow_low_precision(reason="fp16 detection-loss kernel"), \
         TileContext(nc) as tc:
        with (
            tc.tile_pool(name="const", bufs=1) as constp,
            tc.tile_pool(name="anc", bufs=1) as ancp,
            tc.tile_pool(name="pair", bufs=2) as pairp,
            tc.tile_pool(name="clsp", bufs=2) as clsp,
            tc.tile_pool(name="regs", bufs=2) as regsp,
            tc.tile_pool(name="small", bufs=2) as smallp,
            tc.tile_pool(name="ps", bufs=1, space="PSUM") as psp,
            tc.tile_pool(name="psg", bufs=1, space="PSUM") as psgp,
        ):
            def ctile(shape, tag, dtype=f16):
                return constp.tile(shape, dtype, tag=tag, name=tag)

            regb = ctile([128, 4 * NTF], "regb")
            nc.sync.dma_start(regb[:], regb_d[:])
            avv = ctile([128, NTF], "avv")
            nc.sync.dma_start(avv[:], avv_d[:])
            brep = ctile([128, 5 * M * F], "brep")
            nc.sync.dma_start(brep[:], brep_d[:])
            ptab = ctile([128, KQ], "ptab")
            nc.sync.dma_start(ptab[:], ptab_d[:])
            r2b = ctile([128, NT * 256], "r2b")
            nc.sync.dma_start(r2b[:], r2b_d[:])
            s2 = ctile([128, NT * 256], "s2")
            nc.sync.dma_start(s2[:], s2_d[:])

            staging = ctile([128, SC_TOT], "staging", f32)
            nc.vector.memset(staging[:], 0.0)
            se_all = ctile([128, NTF], "se_all")
            cls0_all = ctile([128, NTF], "cls0_all")
            posf_all = ctile([128, NTF], "posf_all")
            negf_all = ctile([128, NTF], "negf_all")

            gmat_ps = psgp.tile([128, 128], f32, tag="gmat_ps", name="gmat_ps")

            # ---- decode (image-wide, fp16 planar) ----
            def aplane(tag):
                return ancp.tile([128, NTF], f16, tag=tag, name=tag)

            dx0, dx1 = aplane("dx0"), aplane("dx1")
            dy0, dy1 = aplane("dy0"), aplane("dy1")
            areaq = aplane("areaq")
            with tc.tile_pool(name="dect", bufs=1) as dtp:
                acap = dtp.tile([128, 4 * NTF], f16, tag="acap", name="acap")
                nc.sync.dma_start(acap[:], acap_d[:])

                def dtile(tag):
                    return dtp.tile([128, NTF], f16, tag=tag, name=tag)

                ewh = dtp.tile([128, 2 * NTF], f16, tag="ewh", name="ewh")
                nc.scalar.activation(ewh[:], regb[:, 2 * NTF:4 * NTF], act.Exp)
                hx, hy = dtile("hx"), dtile("hy")
                nc.vector.tensor_tensor(hx[:], ewh[:, 0:NTF], acap[:, 0:NTF], op.mult)
                nc.vector.tensor_tensor(hy[:], ewh[:, NTF:2 * NTF],
                                        acap[:, 2 * NTF:3 * NTF], op.mult)
                cx, cy = dtile("cx"), dtile("cy")
                nc.vector.tensor_tensor(cx[:], regb[:, 0:NTF], acap[:, 0:NTF], op.mult)
                nc.vector.tensor_tensor(cx[:], cx[:], acap[:, NTF:2 * NTF], op.add)
                nc.vector.tensor_tensor(cy[:], regb[:, NTF:2 * NTF],
                                        acap[:, 2 * NTF:3 * NTF], op.mult)
                nc.vector.tensor_tensor(cy[:], cy[:], acap[:, 3 * NTF:], op.add)
                nc.vector.tensor_tensor(dx0[:], cx[:], hx[:], op.subtract)
                nc.vector.tensor_tensor(dx1[:], cx[:], hx[:], op.add)
                nc.vector.tensor_tensor(dy0[:], cy[:], hy[:], op.subtract)
                nc.vector.tensor_tensor(dy1[:], cy[:], hy[:], op.add)
                nc.vector.tensor_tensor(areaq[:], hx[:], hy[:], op.mult)

            def bplane(i):
                return brep[:, i * M * F:(i + 1) * M * F].rearrange(
                    "p (m f) -> p m f", f=F)

            # ---- per-tile loop (software-pipelined) ----
            state = {}

            def phase1a(t):
                """pair stage through the Ln issues; returns live tiles."""
                fs = slice(t * F, (t + 1) * F)

                def abc(plane):
                    return plane[:, fs].unsqueeze(1).broadcast_to(PMF)

                def ptile(tag, bufs=None):
                    return pairp.tile([128, M * F], f16, tag=tag, name=tag,
                                      bufs=bufs)

                ct = clsp.tile([128, F * C2], f16, tag="ct", name="ct")
                nc.sync.dma_start(ct[:], clsb_d[:, t * F * C2:(t + 1) * F * C2])

                tx1 = ptile("tx1")
                tx1v = tx1[:].rearrange("p (m f) -> p m f", f=F)
                nc.vector.tensor_tensor(tx1v, abc(dx1), bplane(0), op.min)
                nwx = ptile("nwx")
                nwxv = nwx[:].rearrange("p (m f) -> p m f", f=F)
                nc.vector.tensor_tensor(nwxv, abc(dx0), bplane(1), op.max)
                nc.vector.tensor_tensor(nwx[:], nwx[:], tx1[:], op.subtract)
                ty1 = ptile("ty1")
                ty1v = ty1[:].rearrange("p (m f) -> p m f", f=F)
                nc.vector.tensor_tensor(ty1v, abc(dy1), bplane(2), op.min)
                nwy = ptile("nwy")
                nwyv = nwy[:].rearrange("p (m f) -> p m f", f=F)
                nc.vector.tensor_tensor(nwyv, abc(dy0), bplane(3), op.max)
                nc.vector.tensor_tensor(nwy[:], nwy[:], ty1[:], op.subtract)

                ir = ptile("ir", bufs=2)
                nc.vector.scalar_tensor_tensor(ir[:], nwx[:], 0.0, nwy[:],
                                               op.min, op.mult)
                sab = ptile("sab", bufs=2)
                sabv = sab[:].rearrange("p (m f) -> p m f", f=F)
                nc.vector.tensor_tensor(sabv, abc(areaq), bplane(4), op.add)
                nc.vector.tensor_scalar(ir[:], ir[:], CLAMP, None, op.max)
                nc.scalar.activation(ir[:], ir[:], act.Ln)
                nc.scalar.activation(sab[:], sab[:], act.Ln)
                return dict(t=t, ir=ir, sab=sab, ct=ct)

            def phase1b(s):
                """lq, tree-max, masks, transpose, PE gather + gmat, Exp."""
                t = s["t"]
                fs = slice(t * F, (t + 1) * F)
                ir, sab, ct = s["ir"], s["sab"], s["ct"]
                lq = ir
                nc.vector.tensor_tensor(lq[:], lq[:], sab[:], op.subtract)

                def stile(tag, dtype=f16):
                    return smallp.tile([128, F], dtype, tag=tag, name=tag)

                mx = stile("mx")
                tr = pairp.tile([128, M * F // 2], f16, tag="tr", name="tr")
                nc.vector.tensor_tensor(tr[:], lq[:, 0:1024], lq[:, 1024:2048],
                                        op.max)
                nc.vector.tensor_tensor(tr[:, 0:512], tr[:, 0:512],
                                        tr[:, 512:1024], op.max)
                nc.vector.tensor_tensor(tr[:, 0:256], tr[:, 0:256],
                                        tr[:, 256:512], op.max)
                nc.vector.tensor_tensor(tr[:, 0:128], tr[:, 0:128],
                                        tr[:, 128:256], op.max)
                nc.vector.tensor_tensor(mx[:], tr[:, 0:64], tr[:, 64:128],
                                        op.max)
                nc.vector.scalar_tensor_tensor(
                    posf_all[:, fs], mx[:], LN_POS, avv[:, fs], op.is_ge, op.mult,
                    accum_out=staging[:, SC_NP + t:SC_NP + t + 1])
                nc.vector.scalar_tensor_tensor(
                    negf_all[:, fs], mx[:], LN_NEG, avv[:, fs], op.is_lt, op.mult,
                    accum_out=staging[:, SC_NN + t:SC_NN + t + 1])
                bigu = stile("bigu")
                nc.vector.tensor_scalar(bigu[:], posf_all[:, fs], -60000.0,
                                        60000.0, op.mult, op.add)
                mxp = stile("mxp")
                nc.vector.scalar_tensor_tensor(mxp[:], mx[:], -60000.0, bigu[:],
                                               op.max, op.add)

                amask = pairp.tile([128, M * F], f16, tag="amask", name="amask",
                                   bufs=2)
                nc.vector.tensor_tensor(
                    amask[:].rearrange("p (m f) -> p m f", f=F),
                    lq[:].rearrange("p (m f) -> p m f", f=F),
                    mxp[:].unsqueeze(1).broadcast_to(PMF), op.is_ge)
                tmask = pairp.tile([128, M * F], f16, tag="tmask", name="tmask",
                                   bufs=2)
                nc.vector.transpose(
                    tmask[:].rearrange("p (f i) -> p f i", i=32),
                    amask[:].rearrange("p (m f) -> p f m", f=F))

                gps = [psp.tile([128, 16 * KQ], f32, tag=f"g{pb}",
                                name=f"g{pb}") for pb in range(4)]
                for pb in range(4):
                    rows = slice(pb * 32, (pb + 1) * 32)
                    for fc in range(16):
                        nc.tensor.matmul(
                            gps[pb][:, fc * KQ:(fc + 1) * KQ],
                            tmask[rows, fc * 128:(fc + 1) * 128],
                            ptab[rows, :], start=True, stop=True,
                            tile_position=(pb * 32, 0))

                amv2 = amask[:].rearrange("p (m f) -> p f m", f=F)
                for fc in range(16):
                    nc.tensor.matmul(
                        gmat_ps[0:4 * C2, :],
                        ct[:, fc * 4 * C2:(fc + 1) * 4 * C2],
                        amv2[:, 4 * fc:4 * fc + 4, :],
                        start=(t == 0 and fc == 0),
                        stop=(t == NT - 1 and fc == 15))

                et = clsp.tile([128, F * C2], f16, tag="et", name="et")
                nc.scalar.activation(et[:], ct[:], act.Exp)
                s.update(gps=gps, et=et)

            def phase2a(s):
                """evacuate gather PSUM, CE reduce, reg head (through Abs)."""
                t = s["t"]
                fs = slice(t * F, (t + 1) * F)
                gps, et, ct = s["gps"], s["et"], s["ct"]
                ctv = ct[:].rearrange("p (f c) -> p f c", c=C2)

                def stile(tag, dtype=f16):
                    return smallp.tile([128, F], dtype, tag=tag, name=tag)

                def rtile(tag, dtype=f16):
                    return regsp.tile([128, 256], dtype, tag=tag, name=tag)

                g16 = regsp.tile([128, 64 * KQ], f16, tag="g16", name="g16")
                for pb in range(4):
                    nc.vector.tensor_copy(
                        g16[:, pb * 160:(pb + 1) * 160], gps[pb][:])
                gv = g16[:].rearrange("p (ch k) -> p ch k", k=KQ)
                gones = stile("gones")
                nc.vector.tensor_copy(gones[:], gv[:, :, 8])
                wpos = stile("wpos")
                nc.vector.tensor_scalar(wpos[:], gones[:], 0.5, None, op.is_ge)
                recin = stile("recin", f32)
                nc.vector.tensor_scalar(recin[:], gones[:], 1.0, None, op.max)
                wrec = stile("wrec", f32)
                nc.vector.reciprocal(wrec[:], recin[:])

                nc.vector.tensor_reduce(
                    se_all[:, fs], et[:].rearrange("p (f c) -> p f c", c=C2),
                    axis=X, op=op.add)
                nc.vector.tensor_copy(cls0_all[:, fs], ctv[:, :, 0])

                gs = rtile("gs")
                gsv = gs[:].rearrange("p (ch k) -> p ch k", k=4)
                nc.vector.tensor_tensor(gsv, gv[:, :, 0:4], gv[:, :, 4:8],
                                        op.add)
                u = rtile("u")
                nc.vector.tensor_tensor(u[:], gs[:], s2[:, t * 256:(t + 1) * 256],
                                        op.mult)
                uv = u[:].rearrange("p (ch k) -> p ch k", k=4)
                nc.vector.tensor_tensor(
                    uv, uv, wrec[:].unsqueeze(2).broadcast_to([128, F, 4]),
                    op.mult)
                d = rtile("d")
                nc.vector.tensor_tensor(d[:], r2b[:, t * 256:(t + 1) * 256], u[:],
                                        op.subtract)
                ad = rtile("ad")
                nc.scalar.activation(ad[:], d[:], act.Abs)
                s.update(ad=ad, wpos=wpos)

            def phase2b(s):
                """reg tail: smooth-L1 and accumulation."""
                t = s["t"]
                ad, wpos = s["ad"], s["wpos"]

                def rtile(tag, dtype=f16):
                    return regsp.tile([128, 256], dtype, tag=tag, name=tag)

                cc = rtile("cc")
                nc.vector.tensor_scalar(cc[:], ad[:], 1.0, None, op.min)
                t2 = rtile("t2")
                nc.vector.scalar_tensor_tensor(t2[:], cc[:], -0.5, ad[:],
                                               op.mult, op.add)
                q1 = rtile("q1")
                nc.vector.tensor_tensor(q1[:], cc[:], t2[:], op.mult)
                q1w = rtile("q1w")
                nc.vector.scalar_tensor_tensor(
                    q1w[:].rearrange("p (ch k) -> p ch k", k=4),
                    q1[:].rearrange("p (ch k) -> p ch k", k=4), 1.0,
                    wpos[:].unsqueeze(2).broadcast_to([128, F, 4]),
                    op.mult, op.mult,
                    accum_out=staging[:, SC_SL + t:SC_SL + t + 1])

            prev = None
            for t in range(NT):
                cur = phase1a(t)
                if prev is not None:
                    phase2a(prev)
                phase1b(cur)
                if prev is not None:
                    phase2b(prev)
                prev = cur
            phase2a(prev)
            phase2b(prev)

            # ---- final phase ----
            nc.scalar.activation(se_all[:], se_all[:], act.Ln)
            fin = constp.tile([128, NTF], f16, tag="fin", name="fin")
            nc.vector.scalar_tensor_tensor(
                fin[:], posf_all[:], 1.0, se_all[:], op.mult, op.mult,
                accum_out=staging[:, SC_PLSE:SC_PLSE + 1])
            nc.vector.scalar_tensor_tensor(
                fin[:], negf_all[:], 1.0, se_all[:], op.mult, op.mult,
                accum_out=staging[:, SC_NLSE:SC_NLSE + 1])
            nc.vector.scalar_tensor_tensor(
                fin[:], negf_all[:], 1.0, cls0_all[:], op.mult, op.mult,
                accum_out=staging[:, SC_NCLS0:SC_NCLS0 + 1])

            gsb = constp.tile([128, 128], f32, tag="gsb", name="gsb")
            nc.scalar.activation(gsb[:], gmat_ps[:], act.Copy)
            nc.sync.dma_start(gmat_d[:], gsb[:])
            nc.sync.dma_start(stage_d[:], staging[:])

    nc.compile()
    return nc


def _combine(stages, gmats, labels_list):
    cls_losses, reg_losses, n_pos_list = [], [], []
    for st, gm, labs in zip(stages, gmats, labels_list):
        s = st.astype(np.float64)
        n_pos = s[:, SC_NP:SC_NP + NT].sum()
        n_neg = s[:, SC_NN:SC_NN + NT].sum()
        psl1 = s[:, SC_SL:SC_SL + NT].sum()
        plse = s[:, SC_PLSE].sum()
        nlse = s[:, SC_NLSE].sum()
        ncls0 = s[:, SC_NCLS0].sum()
        gm = gm.astype(np.float64)
        G = np.zeros((M, C))
        for fr in range(4):
            G += gm[fr * C2:fr * C2 + C, fr * 32:(fr + 1) * 32].T
        pcls = G[np.arange(M), labs].sum()
        cl = (plse - pcls) / max(n_pos, 1) + (nlse - ncls0) / max(n_neg, 1)
        rl = psl1 / max(4 * n_pos, 1)
        cls_losses.append(cl)
        reg_losses.append(rl)
        n_pos_list.append(n_pos)
    total_pos = int(round(sum(n_pos_list)))
    cls_final = f32n(np.mean(np.array(cls_losses)))
    reg_final = f32n(np.sum(np.array(reg_losses)) / max(total_pos, 1))
    total = f32n(cls_final + reg_final)
    return total, cls_final, reg_final, np.int32(total_pos)


def kernel(cls_output, reg_output, anchors, target_boxes, target_labels):
    global LAST_RESULTS
    import os
    from concourse.bass_utils import run_bass_kernel_spmd

    if "nc" not in _CACHE:
        _CACHE["nc"] = _build_nc()
    nc = _CACHE["nc"]

    acap, avv, s2p, bk, nm = _host_prep_shared(anchors)
    in_maps = []
    for i in range(B):
        regb, r2bp, clsb, brep, ptab = _host_prep_image(
            cls_output[i], reg_output[i], target_boxes[i], nm, bk)
        in_maps.append(dict(regb=regb, acap=acap, avv=avv, brep=brep,
                            clsb=clsb, ptab=ptab, r2b=r2bp, s2=s2p))

    trace = os.environ.get("DETLOSS_TRACE", "0") == "1"
    res = run_bass_kernel_spmd(nc, in_maps, core_ids=list(range(B)), trace=trace)
    LAST_RESULTS = res
    stages = [r["stage"] for r in res.results]
    gmats = [r["gmat"] for r in res.results]
    labels_f = [np.asarray(target_labels[i]).astype(np.int64) for i in range(B)]
    return _combine(stages, gmats, labels_f)


if __name__ == "__main__":
    data = np.load("/root/problem/ref_inputs.npz")
    out = kernel(data["cls_output"], data["reg_output"], data["anchors"],
                 data["target_boxes"], data["target_labels"])
    print("kernel out:", [float(o) for o in out])


# revision 3
# speedup vs baseline: 1.0060x; 1.0060x over previous
"""Trainium2 Bass kernel for the detection-loss problem — v2 (fp16).

Data-parallel over batch: each of 8 NeuronCores processes one image.

Key design vs v1:
  * fp16 everywhere on the pair stage -> DVE 2x packed mode.
  * pair layout [128p, (m=32 boxes outer, f=64 anchors inner)]: every
    broadcast operand ([p, f]-shaped anchor columns, box constants
    replicated host-side) is step-1 in the innermost dim, keeping 2x.
  * ln-space IoU score (monotone): lq = Ln(ir) - Ln(areaq + areaB/4),
    thresholds ln(0.8), ln(4/11). Scalar engine does the two Ln's.
  * argmax one-hot mask -> strided-view 32x32 DVE block transpose gives
    tmask[m-on-partitions, (f, i)] directly; payload gather is 64
    fp16 matmuls/tile (32-row weights, 128-col out) instead of v1's 96
    fp32 (2-pass) 32x32 matmuls; matched-label logits via 16 batched
    [128x128]x[128x88] matmuls/tile accumulating a G[(fr,m),(fr,c)]
    PSUM block (host sums the 4 diagonal blocks).
  * payload split hi/lo fp16 for full coordinate precision; argmax ties
    (~0.3% of anchors) are averaged via a reciprocal of the gathered
    mask-count.
  * per-anchor lse/cls0 stored per tile; single Ln + 3 accumulating
    ops at the end (keeps scalar-engine activation-table swaps at 2/tile).

Anchor indexing: n = t*8192 + p*64 + f  (t tile, p partition, f free),
N 90000 padded to NT*128*F = 90112 with valid=0 rows.
Reg/gather work runs in the matmul-output permuted layout:
  chunk CH = pb*16 + fc, out partition q = f_rel*32 + i:
  anchor n = t*8192 + (pb*32 + i)*64 + (4*fc + f_rel).
"""

import numpy as np

B, A, C, H, W = 8, 9, 21, 100, 100
M = 32
N = A * H * W            # 90000
F = 64                   # anchors per partition per tile
NT = 11
TPB = 128 * F            # 8192
NPAD = NT * TPB          # 90112
NTF = NT * F             # 704
C2 = 22                  # classes padded to 22 for alignment
KQ = 10                  # gather payload columns (hi4, lo4, ones, pad)
LN_POS = float(np.log(0.8))       # iou >= 0.25  <=>  lq >= ln(0.8)
LN_NEG = float(np.log(4.0 / 11.0))  # iou < 0.1  <=>  lq < ln(4/11)
CLAMP = 6.1e-5

# staging columns
SC_NP, SC_NN, SC_SL = 0, NT, 2 * NT
SC_PLSE, SC_NLSE, SC_NCLS0 = 3 * NT, 3 * NT + 1, 3 * NT + 2
SC_TOT = 3 * NT + 3      # 36

_CACHE = {}
LAST_RESULTS = None

f16n = np.float16
f32n = np.float32


def _plane(v):
    """[NPAD] -> [128, NTF] tiling, n = t*TPB + p*F + f."""
    return v.reshape(NT, 128, F).transpose(1, 0, 2).reshape(128, NTF)


def _pad(col, pv):
    full = np.full(NPAD, pv, np.float64)
    full[:N] = col
    return full


def _nmap():
    """[128, NT, 4, 16] anchor index for permuted (matmul-output) layout."""
    q = np.arange(128)[:, None, None, None]
    t = np.arange(NT)[None, :, None, None]
    pb = np.arange(4)[None, None, :, None]
    fc = np.arange(16)[None, None, None, :]
    return t * TPB + (pb * 32 + q % 32) * F + 4 * fc + q // 32


def _host_prep_shared(anchors):
    anchors = np.asarray(anchors, np.float64)
    aw = anchors[:, 2] - anchors[:, 0]
    ah = anchors[:, 3] - anchors[:, 1]
    acx = anchors[:, 0] + 0.5 * aw
    acy = anchors[:, 1] + 0.5 * ah

    acap = np.concatenate([
        _plane(_pad(aw / 2, 1.0)), _plane(_pad(acx - aw / 4, -4000.0)),
        _plane(_pad(ah / 2, 1.0)), _plane(_pad(acy - ah / 4, -4000.0)),
    ], axis=1).astype(f16n).copy()
    avv = _plane(_pad(np.ones(N), 0.0)).astype(f16n).copy()

    s2k = np.stack([_pad(2 / aw, 1.0), _pad(2 / ah, 1.0),
                    _pad(np.ones(N), 1.0), _pad(np.ones(N), 1.0)], 1)
    bk = np.stack([_pad(0.5 - 2 * acx / aw, 0.0), _pad(0.5 - 2 * acy / ah, 0.0),
                   _pad(-np.log(aw), 0.0), _pad(-np.log(ah), 0.0)], 1)
    nm = _nmap()
    s2p = s2k[nm].reshape(128, NT * 256).astype(f16n).copy()
    return acap, avv, s2p, bk, nm


def _host_prep_image(cls_i, reg_i, tb_i, nm, bk):
    cls_flat = np.transpose(np.asarray(cls_i, np.float64), (0, 2, 3, 1)).reshape(N, C)
    reg_flat = np.transpose(np.asarray(reg_i, np.float64), (0, 2, 3, 1)).reshape(N, 4)

    regp = np.zeros((NPAD, 4))
    regp[:N] = reg_flat
    regb = np.concatenate([_plane(regp[:, k]) for k in range(4)], 1).astype(f16n).copy()
    r2bp = (regp - bk)[nm].reshape(128, NT * 256).astype(f16n).copy()

    clsp = np.full((NPAD, C2), -60000.0)
    clsp[:N, :C] = cls_flat
    clsp[N:, :C] = 0.0
    clsb = clsp.reshape(NT, 128, F * C2).transpose(1, 0, 2).reshape(128, -1)
    clsb = clsb.astype(f16n).copy()

    tb = np.asarray(tb_i, np.float64)
    bw = tb[:, 2] - tb[:, 0]
    bh = tb[:, 3] - tb[:, 1]
    planes = [tb[:, 2], tb[:, 0], tb[:, 3], tb[:, 1], bw * bh / 4]
    brep = np.concatenate(
        [np.tile(np.repeat(p, F), (128, 1)) for p in planes], 1).astype(f16n).copy()

    pay = np.stack([tb[:, 0] + bw / 2, tb[:, 1] + bh / 2,
                    np.log(bw), np.log(bh)], 1)       # [M, 4]
    hi = pay.astype(f16n)
    lo = (pay - hi.astype(np.float64)).astype(f16n)
    p10 = np.concatenate([hi, lo, np.ones((M, 1), f16n),
                          np.zeros((M, 1), f16n)], 1)  # [32, 10]
    ptab = np.tile(p10, (4, 1)).astype(f16n).copy()    # [128, 10]
    return regb, r2bp, clsb, brep, ptab


def _build_nc():
    import concourse.bacc as bacc
    import concourse.mybir as mybir
    from concourse.tile import TileContext

    dt = mybir.dt
    f16 = dt.float16
    f32 = dt.float32
    op = mybir.AluOpType
    act = mybir.ActivationFunctionType
    X = mybir.AxisListType.X

    nc = bacc.Bacc("TRN2", target_bir_lowering=False, debug=False, num_devices=8)

    regb_d = nc.dram_tensor("regb", [128, 4 * NTF], f16, kind="ExternalInput")
    acap_d = nc.dram_tensor("acap", [128, 4 * NTF], f16, kind="ExternalInput")
    avv_d = nc.dram_tensor("avv", [128, NTF], f16, kind="ExternalInput")
    brep_d = nc.dram_tensor("brep", [128, 5 * M * F], f16, kind="ExternalInput")
    clsb_d = nc.dram_tensor("clsb", [128, NTF * C2], f16, kind="ExternalInput")
    ptab_d = nc.dram_tensor("ptab", [128, KQ], f16, kind="ExternalInput")
    r2b_d = nc.dram_tensor("r2b", [128, NT * 256], f16, kind="ExternalInput")
    s2_d = nc.dram_tensor("s2", [128, NT * 256], f16, kind="ExternalInput")
    stage_d = nc.dram_tensor("stage", [128, SC_TOT], f32, kind="ExternalOutput")
    gmat_d = nc.dram_tensor("gmat", [128, 128], f32, kind="ExternalOutput")

    PMF = [128, M, F]

    with nc.allow_low_precision(reason="fp16 detection-loss kernel"), \
         TileContext(nc) as tc:
        with (
            tc.tile_pool(name="const", bufs=1) as constp,
            tc.tile_pool(name="anc", bufs=1) as ancp,
            tc.tile_pool(name="pair", bufs=2) as pairp,
            tc.tile_pool(name="clsp", bufs=2) as clsp,
            tc.tile_pool(name="regs", bufs=2) as regsp,
            tc.tile_pool(name="small", bufs=2) as smallp,
            tc.tile_pool(name="ps", bufs=1, space="PSUM") as psp,
            tc.tile_pool(name="psg", bufs=1, space="PSUM") as psgp,
        ):
            def ctile(shape, tag, dtype=f16):
                return constp.tile(shape, dtype, tag=tag, name=tag)

            regb = ctile([128, 4 * NTF], "regb")
            nc.sync.dma_start(regb[:], regb_d[:])
            brep = ctile([128, 5 * M * F], "brep")
            nc.sync.dma_start(brep[:], brep_d[:])
            avv = ctile([128, NTF], "avv")
            nc.sync.dma_start(avv[:], avv_d[:])
            ptab = ctile([128, KQ], "ptab")
            nc.sync.dma_start(ptab[:], ptab_d[:])
            r2b = ctile([128, NT * 256], "r2b")
            s2 = ctile([128, NT * 256], "s2")

            staging = ctile([128, SC_TOT], "staging", f32)
            nc.vector.memset(staging[:], 0.0)
            clampb = ctile([128, 1], "clampb", f32)
            nc.vector.memset(clampb[:], CLAMP)
            se_all = ctile([128, NTF], "se_all")
            cls0_all = ctile([128, NTF], "cls0_all")
            posf_all = ctile([128, NTF], "posf_all")
            negf_all = ctile([128, NTF], "negf_all")

            gmat_ps = psgp.tile([128, 128], f32, tag="gmat_ps", name="gmat_ps")

            # ---- decode (image-wide, fp16 planar) ----
            def aplane(tag):
                return ancp.tile([128, NTF], f16, tag=tag, name=tag)

            dx0, dx1 = aplane("dx0"), aplane("dx1")
            dy0, dy1 = aplane("dy0"), aplane("dy1")
            areaq = aplane("areaq")
            with tc.tile_pool(name="dect", bufs=1) as dtp:
                acap = dtp.tile([128, 4 * NTF], f16, tag="acap", name="acap")
                nc.sync.dma_start(acap[:], acap_d[:])

                def dtile(tag):
                    return dtp.tile([128, NTF], f16, tag=tag, name=tag)

                ewh = dtp.tile([128, 2 * NTF], f16, tag="ewh", name="ewh")
                nc.scalar.activation(ewh[:], regb[:, 2 * NTF:4 * NTF], act.Exp)
                hx, hy = dtile("hx"), dtile("hy")
                nc.vector.tensor_tensor(hx[:], ewh[:, 0:NTF], acap[:, 0:NTF], op.mult)
                nc.vector.tensor_tensor(hy[:], ewh[:, NTF:2 * NTF],
                                        acap[:, 2 * NTF:3 * NTF], op.mult)
                cx, cy = dtile("cx"), dtile("cy")
                nc.vector.tensor_tensor(cx[:], regb[:, 0:NTF], acap[:, 0:NTF], op.mult)
                nc.vector.tensor_tensor(cx[:], cx[:], acap[:, NTF:2 * NTF], op.add)
                nc.vector.tensor_tensor(cy[:], regb[:, NTF:2 * NTF],
                                        acap[:, 2 * NTF:3 * NTF], op.mult)
                nc.vector.tensor_tensor(cy[:], cy[:], acap[:, 3 * NTF:], op.add)
                nc.vector.tensor_tensor(dx0[:], cx[:], hx[:], op.subtract)
                nc.vector.tensor_tensor(dx1[:], cx[:], hx[:], op.add)
                nc.vector.tensor_tensor(dy0[:], cy[:], hy[:], op.subtract)
                nc.vector.tensor_tensor(dy1[:], cy[:], hy[:], op.add)
                nc.vector.tensor_tensor(areaq[:], hx[:], hy[:], op.mult)

            nc.sync.dma_start(r2b[:], r2b_d[:])
            nc.sync.dma_start(s2[:], s2_d[:])

            def bplane(i):
                return brep[:, i * M * F:(i + 1) * M * F].rearrange(
                    "p (m f) -> p m f", f=F)

            # ---- per-tile loop (software-pipelined) ----
            state = {}

            def phase1a(t):
                """pair stage through the Ln issues; returns live tiles."""
                fs = slice(t * F, (t + 1) * F)

                def abc(plane):
                    return plane[:, fs].unsqueeze(1).broadcast_to(PMF)

                def ptile(tag, bufs=None):
                    return pairp.tile([128, M * F], f16, tag=tag, name=tag,
                                      bufs=bufs)

                ct = clsp.tile([128, F * C2], f16, tag="ct", name="ct")
                nc.sync.dma_start(ct[:], clsb_d[:, t * F * C2:(t + 1) * F * C2])

                tx1 = ptile("tx1")
                tx1v = tx1[:].rearrange("p (m f) -> p m f", f=F)
                nc.vector.tensor_tensor(tx1v, abc(dx1), bplane(0), op.min)
                nwx = ptile("nwx")
                nwxv = nwx[:].rearrange("p (m f) -> p m f", f=F)
                nc.vector.tensor_tensor(nwxv, abc(dx0), bplane(1), op.max)
                nc.vector.tensor_tensor(nwx[:], nwx[:], tx1[:], op.subtract)
                ty1 = ptile("ty1")
                ty1v = ty1[:].rearrange("p (m f) -> p m f", f=F)
                nc.vector.tensor_tensor(ty1v, abc(dy1), bplane(2), op.min)
                nwy = ptile("nwy")
                nwyv = nwy[:].rearrange("p (m f) -> p m f", f=F)
                nc.vector.tensor_tensor(nwyv, abc(dy0), bplane(3), op.max)
                nc.vector.tensor_tensor(nwy[:], ty1[:], nwy[:], op.subtract)

                # ox = Relu(-nwx), oy = Relu(nwy) on the scalar engine:
                # makes ir a 2x-mode multiply and ir >= 0 so the clamp folds
                # into Ln's bias
                nc.scalar.activation(nwx[:], nwx[:], act.Relu, scale=-1.0)
                nc.scalar.activation(nwy[:], nwy[:], act.Relu)
                sab = ptile("sab", bufs=2)
                sabv = sab[:].rearrange("p (m f) -> p m f", f=F)
                nc.vector.tensor_tensor(sabv, abc(areaq), bplane(4), op.add)
                return dict(t=t, nwx=nwx, nwy=nwy, sab=sab, ct=ct)

            def phase1b(s):
                """ir product + the two Ln issues."""
                ir = pairp.tile([128, M * F], f16, tag="ir", name="ir", bufs=2)
                nc.vector.tensor_tensor(ir[:], s["nwx"][:], s["nwy"][:],
                                        op.mult)
                nc.scalar.activation(ir[:], ir[:], act.Ln, bias=clampb[0:128, 0:1])
                nc.scalar.activation(s["sab"][:], s["sab"][:], act.Ln)
                s["ir"] = ir

            def phase1c(s):
                """lq, tree-max, masks, transpose, PE gather + gmat, Exp."""
                t = s["t"]
                fs = slice(t * F, (t + 1) * F)
                ir, sab, ct = s["ir"], s["sab"], s["ct"]
                lq = ir
                nc.vector.tensor_tensor(lq[:], lq[:], sab[:], op.subtract)

                def stile(tag, dtype=f16):
                    return smallp.tile([128, F], dtype, tag=tag, name=tag)

                mx = stile("mx")
                tr = pairp.tile([128, M * F // 2], f16, tag="tr", name="tr")
                nc.vector.tensor_tensor(tr[:], lq[:, 0:1024], lq[:, 1024:2048],
                                        op.max)
                nc.vector.tensor_tensor(tr[:, 0:512], tr[:, 0:512],
                                        tr[:, 512:1024], op.max)
                nc.vector.tensor_tensor(tr[:, 0:256], tr[:, 0:256],
                                        tr[:, 256:512], op.max)
                nc.vector.tensor_tensor(tr[:, 0:128], tr[:, 0:128],
                                        tr[:, 128:256], op.max)
                nc.vector.tensor_tensor(mx[:], tr[:, 0:64], tr[:, 64:128],
                                        op.max)
                nc.vector.scalar_tensor_tensor(
                    posf_all[:, fs], mx[:], LN_POS, avv[:, fs], op.is_ge, op.mult,
                    accum_out=staging[:, SC_NP + t:SC_NP + t + 1])
                nc.vector.scalar_tensor_tensor(
                    negf_all[:, fs], mx[:], LN_NEG, avv[:, fs], op.is_lt, op.mult,
                    accum_out=staging[:, SC_NN + t:SC_NN + t + 1])
                bigu = stile("bigu")
                nc.vector.tensor_scalar(bigu[:], posf_all[:, fs], -60000.0,
                                        60000.0, op.mult, op.add)
                mxp = stile("mxp")
                nc.vector.scalar_tensor_tensor(mxp[:], mx[:], -60000.0, bigu[:],
                                               op.max, op.add)

                amask = pairp.tile([128, M * F], f16, tag="amask", name="amask",
                                   bufs=2)
                nc.vector.tensor_tensor(
                    amask[:].rearrange("p (m f) -> p m f", f=F),
                    lq[:].rearrange("p (m f) -> p m f", f=F),
                    mxp[:].unsqueeze(1).broadcast_to(PMF), op.is_ge)
                tmask = pairp.tile([128, M * F], f16, tag="tmask", name="tmask",
                                   bufs=2)
                nc.vector.transpose(
                    tmask[:].rearrange("p (f i) -> p f i", i=32),
                    amask[:].rearrange("p (m f) -> p f m", f=F))

                gps = [psp.tile([128, 16 * KQ], f32, tag=f"g{pb}",
                                name=f"g{pb}") for pb in range(4)]
                for pb in range(4):
                    rows = slice(pb * 32, (pb + 1) * 32)
                    for fc in range(16):
                        nc.tensor.matmul(
                            gps[pb][:, fc * KQ:(fc + 1) * KQ],
                            tmask[rows, fc * 128:(fc + 1) * 128],
                            ptab[rows, :], start=True, stop=True,
                            tile_position=(pb * 32, 0))

                amv2 = amask[:].rearrange("p (m f) -> p f m", f=F)
                for fc in range(16):
                    nc.tensor.matmul(
                        gmat_ps[0:4 * C2, :],
                        ct[:, fc * 4 * C2:(fc + 1) * 4 * C2],
                        amv2[:, 4 * fc:4 * fc + 4, :],
                        start=(t == 0 and fc == 0),
                        stop=(t == NT - 1 and fc == 15))

                et = clsp.tile([128, F * C2], f16, tag="et", name="et")
                nc.scalar.activation(et[:], ct[:], act.Exp)
                s.update(gps=gps, et=et)

            def phase2a(s):
                """evacuate gather PSUM, CE reduce, reg head (through Abs)."""
                t = s["t"]
                fs = slice(t * F, (t + 1) * F)
                gps, et, ct = s["gps"], s["et"], s["ct"]
                ctv = ct[:].rearrange("p (f c) -> p f c", c=C2)

                def stile(tag, dtype=f16):
                    return smallp.tile([128, F], dtype, tag=tag, name=tag)

                def rtile(tag, dtype=f16):
                    return regsp.tile([128, 256], dtype, tag=tag, name=tag)

                g16 = regsp.tile([128, 64 * KQ], f16, tag="g16", name="g16")
                for pb in range(4):
                    nc.vector.tensor_copy(
                        g16[:, pb * 160:(pb + 1) * 160], gps[pb][:])
                gv = g16[:].rearrange("p (ch k) -> p ch k", k=KQ)
                gones = stile("gones")
                nc.vector.tensor_copy(gones[:], gv[:, :, 8])
                wpos = stile("wpos")
                nc.vector.tensor_scalar(wpos[:], gones[:], 0.5, None, op.is_ge)
                recin = stile("recin", f32)
                nc.vector.tensor_scalar(recin[:], gones[:], 1.0, None, op.max)
                wrec = stile("wrec", f32)
                nc.vector.reciprocal(wrec[:], recin[:])

                nc.vector.tensor_reduce(
                    se_all[:, fs],
                    et[:].rearrange("p (f c) -> p f c", c=C2),
                    axis=X, op=op.add)
                gs = rtile("gs")
                gsv = gs[:].rearrange("p (ch k) -> p ch k", k=4)
                nc.vector.tensor_tensor(gsv, gv[:, :, 0:4], gv[:, :, 4:8],
                                        op.add)
                u = rtile("u")
                nc.vector.tensor_tensor(u[:], gs[:], s2[:, t * 256:(t + 1) * 256],
                                        op.mult)
                uv = u[:].rearrange("p (ch k) -> p ch k", k=4)
                nc.vector.tensor_tensor(
                    uv, uv, wrec[:].unsqueeze(2).broadcast_to([128, F, 4]),
                    op.mult)
                d = rtile("d")
                nc.vector.tensor_tensor(d[:], r2b[:, t * 256:(t + 1) * 256], u[:],
                                        op.subtract)
                ad = rtile("ad")
                nc.scalar.activation(ad[:], d[:], act.Abs)
                s.update(ad=ad, wpos=wpos)

            def phase2b(s):
                """CE reduce + reg tail: smooth-L1 and accumulation."""
                t = s["t"]
                fs = slice(t * F, (t + 1) * F)
                ad, wpos = s["ad"], s["wpos"]
                ct = s["ct"]
                nc.vector.tensor_copy(
                    cls0_all[:, fs],
                    ct[:].rearrange("p (f c) -> p f c", c=C2)[:, :, 0])

                def rtile(tag, dtype=f16):
                    return regsp.tile([128, 256], dtype, tag=tag, name=tag)

                cc = rtile("cc")
                nc.vector.tensor_scalar(cc[:], ad[:], 1.0, None, op.min)
                t2 = rtile("t2")
                nc.vector.scalar_tensor_tensor(t2[:], cc[:], -0.5, ad[:],
                                               op.mult, op.add)
                q1 = rtile("q1")
                nc.vector.tensor_tensor(q1[:], cc[:], t2[:], op.mult)
                q1w = rtile("q1w")
                nc.vector.scalar_tensor_tensor(
                    q1w[:].rearrange("p (ch k) -> p ch k", k=4),
                    q1[:].rearrange("p (ch k) -> p ch k", k=4), 1.0,
                    wpos[:].unsqueeze(2).broadcast_to([128, F, 4]),
                    op.mult, op.mult,
                    accum_out=staging[:, SC_SL + t:SC_SL + t + 1])

            prev = None
            for t in range(NT):
                cur = phase1a(t)
                if prev is not None:
                    phase2a(prev)
                phase1b(cur)
                if prev is not None:
                    phase2b(prev)
                phase1c(cur)
                prev = cur
            phase2a(prev)
            phase2b(prev)

            # ---- final phase ----
            nc.scalar.activation(se_all[:], se_all[:], act.Ln)
            fin = constp.tile([128, NTF], f16, tag="fin", name="fin")
            nc.vector.scalar_tensor_tensor(
                fin[:], posf_all[:], 1.0, se_all[:], op.mult, op.mult,
                accum_out=staging[:, SC_PLSE:SC_PLSE + 1])
            nc.vector.scalar_tensor_tensor(
                fin[:], negf_all[:], 1.0, se_all[:], op.mult, op.mult,
                accum_out=staging[:, SC_NLSE:SC_NLSE + 1])
            nc.vector.scalar_tensor_tensor(
                fin[:], negf_all[:], 1.0, cls0_all[:], op.mult, op.mult,
                accum_out=staging[:, SC_NCLS0:SC_NCLS0 + 1])

            gsb = constp.tile([128, 128], f32, tag="gsb", name="gsb")
            nc.scalar.activation(gsb[:], gmat_ps[:], act.Copy)
            nc.sync.dma_start(gmat_d[:], gsb[:])
            nc.sync.dma_start(stage_d[:], staging[:])

    nc.compile()
    return nc


def _combine(stages, gmats, labels_list):
    cls_losses, reg_losses, n_pos_list = [], [], []
    for st, gm, labs in zip(stages, gmats, labels_list):
        s = st.astype(np.float64)
        n_pos = s[:, SC_NP:SC_NP + NT].sum()
        n_neg = s[:, SC_NN:SC_NN + NT].sum()
        psl1 = s[:, SC_SL:SC_SL + NT].sum()
        plse = s[:, SC_PLSE].sum()
        nlse = s[:, SC_NLSE].sum()
        ncls0 = s[:, SC_NCLS0].sum()
        gm = gm.astype(np.float64)
        G = np.zeros((M, C))
        for fr in range(4):
            G += gm[fr * C2:fr * C2 + C, fr * 32:(fr + 1) * 32].T
        pcls = G[np.arange(M), labs].sum()
        cl = (plse - pcls) / max(n_pos, 1) + (nlse - ncls0) / max(n_neg, 1)
        rl = psl1 / max(4 * n_pos, 1)
        cls_losses.append(cl)
        reg_losses.append(rl)
        n_pos_list.append(n_pos)
    total_pos = int(round(sum(n_pos_list)))
    cls_final = f32n(np.mean(np.array(cls_losses)))
    reg_final = f32n(np.sum(np.array(reg_losses)) / max(total_pos, 1))
    total = f32n(cls_final + reg_final)
    return total, cls_final, reg_final, np.int32(total_pos)


def kernel(cls_output, reg_output, anchors, target_boxes, target_labels):
    global LAST_RESULTS
    import os
    from concourse.bass_utils import run_bass_kernel_spmd

    if "nc" not in _CACHE:
        _CACHE["nc"] = _build_nc()
    nc = _CACHE["nc"]

    acap, avv, s2p, bk, nm = _host_prep_shared(anchors)
    in_maps = []
    for i in range(B):
        regb, r2bp, clsb, brep, ptab = _host_prep_image(
            cls_output[i], reg_output[i], target_boxes[i], nm, bk)
        in_maps.append(dict(regb=regb, acap=acap, avv=avv, brep=brep,
                            clsb=clsb, ptab=ptab, r2b=r2bp, s2=s2p))

    trace = os.environ.get("DETLOSS_TRACE", "0") == "1"
    res = run_bass_kernel_spmd(nc, in_maps, core_ids=list(range(B)), trace=trace)
    LAST_RESULTS = res
    stages = [r["stage"] for r in res.results]
    gmats = [r["gmat"] for r in res.results]
    labels_f = [np.asarray(target_labels[i]).astype(np.int64) for i in range(B)]
    return _combine(stages, gmats, labels_f)


if __name__ == "__main__":
    data = np.load("/root/problem/ref_inputs.npz")
    out = kernel(data["cls_output"], data["reg_output"], data["anchors"],
                 data["target_boxes"], data["target_labels"])
    print("kernel out:", [float(o) for o in out])


# revision 4
# speedup vs baseline: 1.0121x; 1.0060x over previous
"""Trainium2 Bass kernel for the detection-loss problem — v2 (fp16).

Data-parallel over batch: each of 8 NeuronCores processes one image.

Key design vs v1:
  * fp16 everywhere on the pair stage -> DVE 2x packed mode.
  * pair layout [128p, (m=32 boxes outer, f=64 anchors inner)]: every
    broadcast operand ([p, f]-shaped anchor columns, box constants
    replicated host-side) is step-1 in the innermost dim, keeping 2x.
  * ln-space IoU score (monotone): lq = Ln(ir) - Ln(areaq + areaB/4),
    thresholds ln(0.8), ln(4/11). Scalar engine does the two Ln's.
  * argmax one-hot mask -> strided-view 32x32 DVE block transpose gives
    tmask[m-on-partitions, (f, i)] directly; payload gather is 64
    fp16 matmuls/tile (32-row weights, 128-col out) instead of v1's 96
    fp32 (2-pass) 32x32 matmuls; matched-label logits via 16 batched
    [128x128]x[128x88] matmuls/tile accumulating a G[(fr,m),(fr,c)]
    PSUM block (host sums the 4 diagonal blocks).
  * payload split hi/lo fp16 for full coordinate precision; argmax ties
    (~0.3% of anchors) are averaged via a reciprocal of the gathered
    mask-count.
  * per-anchor lse/cls0 stored per tile; single Ln + 3 accumulating
    ops at the end (keeps scalar-engine activation-table swaps at 2/tile).

Anchor indexing: n = t*8192 + p*64 + f  (t tile, p partition, f free),
N 90000 padded to NT*128*F = 90112 with valid=0 rows.
Reg/gather work runs in the matmul-output permuted layout:
  chunk CH = pb*16 + fc, out partition q = f_rel*32 + i:
  anchor n = t*8192 + (pb*32 + i)*64 + (4*fc + f_rel).
"""

import numpy as np

B, A, C, H, W = 8, 9, 21, 100, 100
M = 32
N = A * H * W            # 90000
F = 64                   # anchors per partition per tile
NT = 11
TPB = 128 * F            # 8192
NPAD = NT * TPB          # 90112
NTF = NT * F             # 704
C2 = 22                  # classes padded to 22 for alignment
KQ = 10                  # gather payload columns (hi4, lo4, ones, pad)
LN_POS = float(np.log(0.8))       # iou >= 0.25  <=>  lq >= ln(0.8)
LN_NEG = float(np.log(4.0 / 11.0))  # iou < 0.1  <=>  lq < ln(4/11)
CLAMP = 6.1e-5

# staging columns
SC_NP, SC_NN, SC_SL = 0, NT, 2 * NT
SC_PLSE, SC_NLSE, SC_NCLS0 = 3 * NT, 3 * NT + 1, 3 * NT + 2
SC_TOT = 3 * NT + 3      # 36

_CACHE = {}
LAST_RESULTS = None

f16n = np.float16
f32n = np.float32


def _plane(v):
    """[NPAD] -> [128, NTF] tiling, n = t*TPB + p*F + f."""
    return v.reshape(NT, 128, F).transpose(1, 0, 2).reshape(128, NTF)


def _pad(col, pv):
    full = np.full(NPAD, pv, np.float64)
    full[:N] = col
    return full


def _nmap():
    """[128, NT, 4, 16] anchor index for permuted (matmul-output) layout."""
    q = np.arange(128)[:, None, None, None]
    t = np.arange(NT)[None, :, None, None]
    pb = np.arange(4)[None, None, :, None]
    fc = np.arange(16)[None, None, None, :]
    return t * TPB + (pb * 32 + q % 32) * F + 4 * fc + q // 32


def _host_prep_shared(anchors):
    anchors = np.asarray(anchors, np.float64)
    aw = anchors[:, 2] - anchors[:, 0]
    ah = anchors[:, 3] - anchors[:, 1]
    acx = anchors[:, 0] + 0.5 * aw
    acy = anchors[:, 1] + 0.5 * ah

    acap = np.concatenate([
        _plane(_pad(aw / 2, 1.0)), _plane(_pad(acx - aw / 4, -4000.0)),
        _plane(_pad(ah / 2, 1.0)), _plane(_pad(acy - ah / 4, -4000.0)),
    ], axis=1).astype(f16n).copy()
    avv = _plane(_pad(np.ones(N), 0.0)).astype(f16n).copy()

    s2k = np.stack([_pad(2 / aw, 1.0), _pad(2 / ah, 1.0),
                    _pad(np.ones(N), 1.0), _pad(np.ones(N), 1.0)], 1)
    bk = np.stack([_pad(0.5 - 2 * acx / aw, 0.0), _pad(0.5 - 2 * acy / ah, 0.0),
                   _pad(-np.log(aw), 0.0), _pad(-np.log(ah), 0.0)], 1)
    nm = _nmap()
    s2p = s2k[nm].reshape(128, NT * 256).astype(f16n).copy()
    return acap, avv, s2p, bk, nm


def _host_prep_image(cls_i, reg_i, tb_i, nm, bk):
    cls_flat = np.transpose(np.asarray(cls_i, np.float64), (0, 2, 3, 1)).reshape(N, C)
    reg_flat = np.transpose(np.asarray(reg_i, np.float64), (0, 2, 3, 1)).reshape(N, 4)

    regp = np.zeros((NPAD, 4))
    regp[:N] = reg_flat
    regb = np.concatenate([_plane(regp[:, k]) for k in range(4)], 1).astype(f16n).copy()
    r2bp = (regp - bk)[nm].reshape(128, NT * 256).astype(f16n).copy()

    clsp = np.full((NPAD, C2), -60000.0)
    clsp[:N, :C] = cls_flat
    clsp[N:, :C] = 0.0
    clsb = clsp.reshape(NT, 128, F * C2).transpose(1, 0, 2).reshape(128, -1)
    clsb = clsb.astype(f16n).copy()

    tb = np.asarray(tb_i, np.float64)
    bw = tb[:, 2] - tb[:, 0]
    bh = tb[:, 3] - tb[:, 1]
    planes = [tb[:, 2], tb[:, 0], tb[:, 3], tb[:, 1], bw * bh / 4]
    brep = np.concatenate(
        [np.tile(np.repeat(p, F), (128, 1)) for p in planes], 1).astype(f16n).copy()

    pay = np.stack([tb[:, 0] + bw / 2, tb[:, 1] + bh / 2,
                    np.log(bw), np.log(bh)], 1)       # [M, 4]
    hi = pay.astype(f16n)
    lo = (pay - hi.astype(np.float64)).astype(f16n)
    p10 = np.concatenate([hi, lo, np.ones((M, 1), f16n),
                          np.zeros((M, 1), f16n)], 1)  # [32, 10]
    ptab = np.tile(p10, (4, 1)).astype(f16n).copy()    # [128, 10]
    return regb, r2bp, clsb, brep, ptab


def _build_nc():
    import concourse.bacc as bacc
    import concourse.mybir as mybir
    from concourse.tile import TileContext

    dt = mybir.dt
    f16 = dt.float16
    f32 = dt.float32
    op = mybir.AluOpType
    act = mybir.ActivationFunctionType
    X = mybir.AxisListType.X

    nc = bacc.Bacc("TRN2", target_bir_lowering=False, debug=False, num_devices=8)

    regb_d = nc.dram_tensor("regb", [128, 4 * NTF], f16, kind="ExternalInput")
    acap_d = nc.dram_tensor("acap", [128, 4 * NTF], f16, kind="ExternalInput")
    avv_d = nc.dram_tensor("avv", [128, NTF], f16, kind="ExternalInput")
    brep_d = nc.dram_tensor("brep", [128, 5 * M * F], f16, kind="ExternalInput")
    clsb_d = nc.dram_tensor("clsb", [128, NTF * C2], f16, kind="ExternalInput")
    ptab_d = nc.dram_tensor("ptab", [128, KQ], f16, kind="ExternalInput")
    r2b_d = nc.dram_tensor("r2b", [128, NT * 256], f16, kind="ExternalInput")
    s2_d = nc.dram_tensor("s2", [128, NT * 256], f16, kind="ExternalInput")
    stage_d = nc.dram_tensor("stage", [128, SC_TOT], f32, kind="ExternalOutput")
    gmat_d = nc.dram_tensor("gmat", [128, 128], f32, kind="ExternalOutput")

    PMF = [128, M, F]

    with nc.allow_low_precision(reason="fp16 detection-loss kernel"), \
         TileContext(nc) as tc:
        with (
            tc.tile_pool(name="const", bufs=1) as constp,
            tc.tile_pool(name="anc", bufs=1) as ancp,
            tc.tile_pool(name="pair", bufs=2) as pairp,
            tc.tile_pool(name="clsp", bufs=2) as clsp,
            tc.tile_pool(name="regs", bufs=2) as regsp,
            tc.tile_pool(name="small", bufs=2) as smallp,
            tc.tile_pool(name="ps", bufs=1, space="PSUM") as psp,
            tc.tile_pool(name="psg", bufs=1, space="PSUM") as psgp,
        ):
            def ctile(shape, tag, dtype=f16):
                return constp.tile(shape, dtype, tag=tag, name=tag)

            regb = ctile([128, 4 * NTF], "regb")
            nc.sync.dma_start(regb[:], regb_d[:])
            acap = ctile([128, 4 * NTF], "acap")
            nc.sync.dma_start(acap[:], acap_d[:])
            brep = ctile([128, 5 * M * F], "brep")
            nc.sync.dma_start(brep[:], brep_d[:])
            avv = ctile([128, NTF], "avv")
            nc.sync.dma_start(avv[:], avv_d[:])
            ptab = ctile([128, KQ], "ptab")
            nc.sync.dma_start(ptab[:], ptab_d[:])
            r2b = ctile([128, NT * 256], "r2b")
            s2 = ctile([128, NT * 256], "s2")

            staging = ctile([128, SC_TOT], "staging", f32)
            nc.vector.memset(staging[:], 0.0)
            clampb = ctile([128, 1], "clampb", f32)
            nc.vector.memset(clampb[:], CLAMP)
            se_all = ctile([128, NTF], "se_all")
            cls0_all = ctile([128, NTF], "cls0_all")
            posf_all = ctile([128, NTF], "posf_all")
            negf_all = ctile([128, NTF], "negf_all")

            gmat_ps = psgp.tile([128, 128], f32, tag="gmat_ps", name="gmat_ps")

            # ---- decode (image-wide, fp16 planar) ----
            def aplane(tag):
                return ancp.tile([128, NTF], f16, tag=tag, name=tag)

            dx0, dx1 = aplane("dx0"), aplane("dx1")
            dy0, dy1 = aplane("dy0"), aplane("dy1")
            areaq = aplane("areaq")
            with tc.tile_pool(name="dect", bufs=1) as dtp:
                def dtile(tag):
                    return dtp.tile([128, NTF], f16, tag=tag, name=tag)

                ewh = dtp.tile([128, 2 * NTF], f16, tag="ewh", name="ewh")
                nc.scalar.activation(ewh[:], regb[:, 2 * NTF:4 * NTF], act.Exp)
                hx, hy = dtile("hx"), dtile("hy")
                nc.vector.tensor_tensor(hx[:], ewh[:, 0:NTF], acap[:, 0:NTF], op.mult)
                nc.vector.tensor_tensor(hy[:], ewh[:, NTF:2 * NTF],
                                        acap[:, 2 * NTF:3 * NTF], op.mult)
                cx, cy = dtile("cx"), dtile("cy")
                nc.vector.tensor_tensor(cx[:], regb[:, 0:NTF], acap[:, 0:NTF], op.mult)
                nc.vector.tensor_tensor(cx[:], cx[:], acap[:, NTF:2 * NTF], op.add)
                nc.vector.tensor_tensor(cy[:], regb[:, NTF:2 * NTF],
                                        acap[:, 2 * NTF:3 * NTF], op.mult)
                nc.vector.tensor_tensor(cy[:], cy[:], acap[:, 3 * NTF:], op.add)
                nc.vector.tensor_tensor(dx0[:], cx[:], hx[:], op.subtract)
                nc.vector.tensor_tensor(dx1[:], cx[:], hx[:], op.add)
                nc.vector.tensor_tensor(dy0[:], cy[:], hy[:], op.subtract)
                nc.vector.tensor_tensor(dy1[:], cy[:], hy[:], op.add)
                nc.vector.tensor_tensor(areaq[:], hx[:], hy[:], op.mult)

            nc.sync.dma_start(r2b[:], r2b_d[:])
            nc.sync.dma_start(s2[:], s2_d[:])

            def bplane(i):
                return brep[:, i * M * F:(i + 1) * M * F].rearrange(
                    "p (m f) -> p m f", f=F)

            # ---- per-tile loop (software-pipelined) ----
            state = {}

            def phase1a(t):
                """pair stage through the Ln issues; returns live tiles."""
                fs = slice(t * F, (t + 1) * F)

                def abc(plane):
                    return plane[:, fs].unsqueeze(1).broadcast_to(PMF)

                def ptile(tag, bufs=None):
                    return pairp.tile([128, M * F], f16, tag=tag, name=tag,
                                      bufs=bufs)

                ct = clsp.tile([128, F * C2], f16, tag="ct", name="ct")
                nc.sync.dma_start(ct[:], clsb_d[:, t * F * C2:(t + 1) * F * C2])

                tx1 = ptile("tx1")
                tx1v = tx1[:].rearrange("p (m f) -> p m f", f=F)
                nc.vector.tensor_tensor(tx1v, abc(dx1), bplane(0), op.min)
                nwx = ptile("nwx")
                nwxv = nwx[:].rearrange("p (m f) -> p m f", f=F)
                nc.vector.tensor_tensor(nwxv, abc(dx0), bplane(1), op.max)
                nc.vector.tensor_tensor(nwx[:], nwx[:], tx1[:], op.subtract)
                ty1 = ptile("ty1")
                ty1v = ty1[:].rearrange("p (m f) -> p m f", f=F)
                nc.vector.tensor_tensor(ty1v, abc(dy1), bplane(2), op.min)
                nwy = ptile("nwy")
                nwyv = nwy[:].rearrange("p (m f) -> p m f", f=F)
                nc.vector.tensor_tensor(nwyv, abc(dy0), bplane(3), op.max)
                nc.vector.tensor_tensor(nwy[:], ty1[:], nwy[:], op.subtract)

                # ox = Relu(-nwx), oy = Relu(nwy) on the scalar engine:
                # makes ir a 2x-mode multiply and ir >= 0 so the clamp folds
                # into Ln's bias
                nc.scalar.activation(nwx[:], nwx[:], act.Relu, scale=-1.0)
                nc.scalar.activation(nwy[:], nwy[:], act.Relu)
                sab = ptile("sab", bufs=3)
                sabv = sab[:].rearrange("p (m f) -> p m f", f=F)
                nc.vector.tensor_tensor(sabv, abc(areaq), bplane(4), op.add)
                return dict(t=t, nwx=nwx, nwy=nwy, sab=sab, ct=ct)

            def phase1b(s):
                """ir product + the two Ln issues."""
                ir = pairp.tile([128, M * F], f16, tag="ir", name="ir", bufs=3)
                nc.vector.tensor_tensor(ir[:], s["nwx"][:], s["nwy"][:],
                                        op.mult)
                nc.scalar.activation(ir[:], ir[:], act.Ln, bias=clampb[0:128, 0:1])
                nc.scalar.activation(s["sab"][:], s["sab"][:], act.Ln)
                s["ir"] = ir

            def phase1c(s):
                """lq, tree-max, masks, transpose, PE gather + gmat, Exp."""
                t = s["t"]
                fs = slice(t * F, (t + 1) * F)
                ir, sab, ct = s["ir"], s["sab"], s["ct"]
                lq = ir
                nc.vector.tensor_tensor(lq[:], lq[:], sab[:], op.subtract)

                def stile(tag, dtype=f16):
                    return smallp.tile([128, F], dtype, tag=tag, name=tag)

                mx = stile("mx")
                tr = pairp.tile([128, M * F // 2], f16, tag="tr", name="tr")
                nc.vector.tensor_tensor(tr[:], lq[:, 0:1024], lq[:, 1024:2048],
                                        op.max)
                nc.vector.tensor_tensor(tr[:, 0:512], tr[:, 0:512],
                                        tr[:, 512:1024], op.max)
                nc.vector.tensor_tensor(tr[:, 0:256], tr[:, 0:256],
                                        tr[:, 256:512], op.max)
                nc.vector.tensor_tensor(tr[:, 0:128], tr[:, 0:128],
                                        tr[:, 128:256], op.max)
                nc.vector.tensor_tensor(mx[:], tr[:, 0:64], tr[:, 64:128],
                                        op.max)
                nc.vector.scalar_tensor_tensor(
                    posf_all[:, fs], mx[:], LN_POS, avv[:, fs], op.is_ge, op.mult,
                    accum_out=staging[:, SC_NP + t:SC_NP + t + 1])
                nc.vector.scalar_tensor_tensor(
                    negf_all[:, fs], mx[:], LN_NEG, avv[:, fs], op.is_lt, op.mult,
                    accum_out=staging[:, SC_NN + t:SC_NN + t + 1])
                bigu = stile("bigu")
                nc.vector.tensor_scalar(bigu[:], posf_all[:, fs], -60000.0,
                                        60000.0, op.mult, op.add)
                mxp = stile("mxp")
                nc.vector.scalar_tensor_tensor(mxp[:], mx[:], -60000.0, bigu[:],
                                               op.max, op.add)

                amask = pairp.tile([128, M * F], f16, tag="amask", name="amask",
                                   bufs=2)
                nc.vector.tensor_tensor(
                    amask[:].rearrange("p (m f) -> p m f", f=F),
                    lq[:].rearrange("p (m f) -> p m f", f=F),
                    mxp[:].unsqueeze(1).broadcast_to(PMF), op.is_ge)
                tmask = pairp.tile([128, M * F], f16, tag="tmask", name="tmask",
                                   bufs=2)
                nc.vector.transpose(
                    tmask[:].rearrange("p (f i) -> p f i", i=32),
                    amask[:].rearrange("p (m f) -> p f m", f=F))

                gps = [psp.tile([128, 16 * KQ], f32, tag=f"g{pb}",
                                name=f"g{pb}") for pb in range(4)]
                for pb in range(4):
                    rows = slice(pb * 32, (pb + 1) * 32)
                    for fc in range(16):
                        nc.tensor.matmul(
                            gps[pb][:, fc * KQ:(fc + 1) * KQ],
                            tmask[rows, fc * 128:(fc + 1) * 128],
                            ptab[rows, :], start=True, stop=True,
                            tile_position=(pb * 32, 0))

                amv2 = amask[:].rearrange("p (m f) -> p f m", f=F)
                for fc in range(16):
                    nc.tensor.matmul(
                        gmat_ps[0:4 * C2, :],
                        ct[:, fc * 4 * C2:(fc + 1) * 4 * C2],
                        amv2[:, 4 * fc:4 * fc + 4, :],
                        start=(t == 0 and fc == 0),
                        stop=(t == NT - 1 and fc == 15))

                et = clsp.tile([128, F * C2], f16, tag="et", name="et")
                nc.scalar.activation(et[:], ct[:], act.Exp)
                s.update(gps=gps, et=et)

            def phase2a(s):
                """evacuate gather PSUM, CE reduce, reg head (through Abs)."""
                t = s["t"]
                fs = slice(t * F, (t + 1) * F)
                gps, et, ct = s["gps"], s["et"], s["ct"]
                ctv = ct[:].rearrange("p (f c) -> p f c", c=C2)

                def stile(tag, dtype=f16):
                    return smallp.tile([128, F], dtype, tag=tag, name=tag)

                def rtile(tag, dtype=f16):
                    return regsp.tile([128, 256], dtype, tag=tag, name=tag)

                g16 = regsp.tile([128, 64 * KQ], f16, tag="g16", name="g16")
                for pb in range(4):
                    nc.vector.tensor_copy(
                        g16[:, pb * 160:(pb + 1) * 160], gps[pb][:])
                gv = g16[:].rearrange("p (ch k) -> p ch k", k=KQ)
                gones = stile("gones")
                nc.vector.tensor_copy(gones[:], gv[:, :, 8])
                wpos = stile("wpos")
                nc.vector.tensor_scalar(wpos[:], gones[:], 0.5, None, op.is_ge)
                recin = stile("recin", f32)
                nc.vector.tensor_scalar(recin[:], gones[:], 1.0, None, op.max)
                wrec = stile("wrec", f32)
                nc.vector.reciprocal(wrec[:], recin[:])

                nc.vector.tensor_reduce(
                    se_all[:, fs],
                    et[:].rearrange("p (f c) -> p f c", c=C2),
                    axis=X, op=op.add)
                gs = rtile("gs")
                gsv = gs[:].rearrange("p (ch k) -> p ch k", k=4)
                nc.vector.tensor_tensor(gsv, gv[:, :, 0:4], gv[:, :, 4:8],
                                        op.add)
                u = rtile("u")
                nc.vector.tensor_tensor(u[:], gs[:], s2[:, t * 256:(t + 1) * 256],
                                        op.mult)
                uv = u[:].rearrange("p (ch k) -> p ch k", k=4)
                nc.vector.tensor_tensor(
                    uv, uv, wrec[:].unsqueeze(2).broadcast_to([128, F, 4]),
                    op.mult)
                d = rtile("d")
                nc.vector.tensor_tensor(d[:], r2b[:, t * 256:(t + 1) * 256], u[:],
                                        op.subtract)
                nd = rtile("nd")
                nc.vector.tensor_scalar(nd[:], d[:], -1.0, None, op.mult)
                ad = rtile("ad")
                nc.vector.tensor_tensor(ad[:], d[:], nd[:], op.max)
                s.update(ad=ad, wpos=wpos)

            def phase2b(s):
                """CE reduce + reg tail: smooth-L1 and accumulation."""
                t = s["t"]
                fs = slice(t * F, (t + 1) * F)
                ad, wpos = s["ad"], s["wpos"]
                ct = s["ct"]
                nc.vector.tensor_copy(
                    cls0_all[:, fs],
                    ct[:].rearrange("p (f c) -> p f c", c=C2)[:, :, 0])

                def rtile(tag, dtype=f16):
                    return regsp.tile([128, 256], dtype, tag=tag, name=tag)

                cc = rtile("cc")
                nc.vector.tensor_scalar(cc[:], ad[:], 1.0, None, op.min)
                t2 = rtile("t2")
                nc.vector.scalar_tensor_tensor(t2[:], cc[:], -0.5, ad[:],
                                               op.mult, op.add)
                q1 = rtile("q1")
                nc.vector.tensor_tensor(q1[:], cc[:], t2[:], op.mult)
                q1w = rtile("q1w")
                nc.vector.scalar_tensor_tensor(
                    q1w[:].rearrange("p (ch k) -> p ch k", k=4),
                    q1[:].rearrange("p (ch k) -> p ch k", k=4), 1.0,
                    wpos[:].unsqueeze(2).broadcast_to([128, F, 4]),
                    op.mult, op.mult,
                    accum_out=staging[:, SC_SL + t:SC_SL + t + 1])

            prev = None
            for t in range(NT):
                cur = phase1a(t)
                if prev is not None:
                    phase2a(prev)
                phase1b(cur)
                if prev is not None:
                    phase2b(prev)
                phase1c(cur)
                prev = cur
            phase2a(prev)
            phase2b(prev)

            # ---- final phase ----
            nc.scalar.activation(se_all[:], se_all[:], act.Ln)
            fin = constp.tile([128, NTF], f16, tag="fin", name="fin")
            nc.vector.scalar_tensor_tensor(
                fin[:], posf_all[:], 1.0, se_all[:], op.mult, op.mult,
                accum_out=staging[:, SC_PLSE:SC_PLSE + 1])
            nc.vector.scalar_tensor_tensor(
                fin[:], negf_all[:], 1.0, se_all[:], op.mult, op.mult,
                accum_out=staging[:, SC_NLSE:SC_NLSE + 1])
            nc.vector.scalar_tensor_tensor(
                fin[:], negf_all[:], 1.0, cls0_all[:], op.mult, op.mult,
                accum_out=staging[:, SC_NCLS0:SC_NCLS0 + 1])

            gsb = constp.tile([128, 128], f32, tag="gsb", name="gsb")
            nc.scalar.activation(gsb[:], gmat_ps[:], act.Copy)
            nc.sync.dma_start(gmat_d[:], gsb[:])
            nc.sync.dma_start(stage_d[:], staging[:])

    nc.compile()
    return nc


def _combine(stages, gmats, labels_list):
    cls_losses, reg_losses, n_pos_list = [], [], []
    for st, gm, labs in zip(stages, gmats, labels_list):
        s = st.astype(np.float64)
        n_pos = s[:, SC_NP:SC_NP + NT].sum()
        n_neg = s[:, SC_NN:SC_NN + NT].sum()
        psl1 = s[:, SC_SL:SC_SL + NT].sum()
        plse = s[:, SC_PLSE].sum()
        nlse = s[:, SC_NLSE].sum()
        ncls0 = s[:, SC_NCLS0].sum()
        gm = gm.astype(np.float64)
        G = np.zeros((M, C))
        for fr in range(4):
            G += gm[fr * C2:fr * C2 + C, fr * 32:(fr + 1) * 32].T
        pcls = G[np.arange(M), labs].sum()
        cl = (plse - pcls) / max(n_pos, 1) + (nlse - ncls0) / max(n_neg, 1)
        rl = psl1 / max(4 * n_pos, 1)
        cls_losses.append(cl)
        reg_losses.append(rl)
        n_pos_list.append(n_pos)
    total_pos = int(round(sum(n_pos_list)))
    cls_final = f32n(np.mean(np.array(cls_losses)))
    reg_final = f32n(np.sum(np.array(reg_losses)) / max(total_pos, 1))
    total = f32n(cls_final + reg_final)
    return total, cls_final, reg_final, np.int32(total_pos)


def kernel(cls_output, reg_output, anchors, target_boxes, target_labels):
    global LAST_RESULTS
    import os
    from concourse.bass_utils import run_bass_kernel_spmd

    if "nc" not in _CACHE:
        _CACHE["nc"] = _build_nc()
    nc = _CACHE["nc"]

    acap, avv, s2p, bk, nm = _host_prep_shared(anchors)
    in_maps = []
    for i in range(B):
        regb, r2bp, clsb, brep, ptab = _host_prep_image(
            cls_output[i], reg_output[i], target_boxes[i], nm, bk)
        in_maps.append(dict(regb=regb, acap=acap, avv=avv, brep=brep,
                            clsb=clsb, ptab=ptab, r2b=r2bp, s2=s2p))

    trace = os.environ.get("DETLOSS_TRACE", "0") == "1"
    res = run_bass_kernel_spmd(nc, in_maps, core_ids=list(range(B)), trace=trace)
    LAST_RESULTS = res
    stages = [r["stage"] for r in res.results]
    gmats = [r["gmat"] for r in res.results]
    labels_f = [np.asarray(target_labels[i]).astype(np.int64) for i in range(B)]
    return _combine(stages, gmats, labels_f)


if __name__ == "__main__":
    data = np.load("/root/problem/ref_inputs.npz")
    out = kernel(data["cls_output"], data["reg_output"], data["anchors"],
                 data["target_boxes"], data["target_labels"])
    print("kernel out:", [float(o) for o in out])
